# revision 2
# baseline (speedup 1.0000x reference)
"""Trainium2 Bass kernel for unscaled cross-attention (key doubles as value).

Problem: B=8, Tq=Tk=2048, D=1024, fp32.
  energy = Q @ K^T  ->  softmax over Tk  ->  out = attn @ K

Sharding: batch dim across the 8 NeuronCores (1 batch element per core).

Per-core algorithm (all matmuls in float32r — fp32 storage, mantissa rounded
to ~12 bits, runs at full PE rate):
  prologue: stream K in 16 row-chunks; build K^T [d,k] via PE transposes and
            K natural [k,d] (rounded to f32r), both resident in SBUF.
  per q-block (128 rows, 16 blocks):
    load Q block, PE-transpose to Q^T [d,q] (f32r)
    S = Q^T.T @ K^T          (PSUM [128, 2048], accumulate over 8 d-tiles)
    negmax = -rowmax(S)       (DVE)
    P = exp(S + negmax), sumexp = rowsum (fused, ACT)
    P^T via PE transposes     (f32r)
    O = P^T.T @ K_nat         (PSUM [128, 1024], accumulate over 16 k-tiles)
    out = O * (1/sumexp)      (DVE), DMA to DRAM
"""

import sys

if "/opt/trn_rl_repo" not in sys.path:
    sys.path.insert(0, "/opt/trn_rl_repo")

import numpy as np

import concourse.bacc as bacc
import concourse.tile as tile
from concourse import mybir
from concourse.bass_utils import run_bass_kernel_spmd
from concourse.masks import make_identity

N_CORES = 8
T = 2048          # Tq == Tk
D = 1024
P = 128
DO = D // P       # 8 d-tiles
KO = T // P       # 16 k-tiles
QB = T // P       # 16 q-blocks
F32 = mybir.dt.float32
F32R = mybir.dt.float32r


def build_body(nc, tc, ctx, q_ap, k_ap, out_ap, n_reps=1):
    const = ctx.enter_context(tc.tile_pool(name="const", bufs=1))
    kt_pool = ctx.enter_context(tc.tile_pool(name="kt", bufs=1))
    knat_pool = ctx.enter_context(tc.tile_pool(name="knat", bufs=1))
    ld_pool = ctx.enter_context(tc.tile_pool(name="ld", bufs=3))
    qt_pool = ctx.enter_context(tc.tile_pool(name="qt", bufs=2))
    p_pool = ctx.enter_context(tc.tile_pool(name="p", bufs=2))
    pt_pool = ctx.enter_context(tc.tile_pool(name="pt", bufs=1))
    o_pool = ctx.enter_context(tc.tile_pool(name="o", bufs=2))
    stat_pool = ctx.enter_context(tc.tile_pool(name="stat", bufs=4))
    s_psum = ctx.enter_context(tc.tile_pool(name="s_ps", bufs=1, space="PSUM"))
    tr_psum = ctx.enter_context(tc.tile_pool(name="tr_ps", bufs=2, space="PSUM"))
    o_psum = ctx.enter_context(tc.tile_pool(name="o_ps", bufs=1, space="PSUM"))

    ident = const.tile([P, P], F32)
    make_identity(nc, ident)

    kt = kt_pool.tile([P, DO, T], F32R)       # kt[dd, do, k] = K[k, do*128+dd]
    knat = knat_pool.tile([P, KO, D], F32R)   # knat[kk, ko, d] = K[ko*128+kk, d]

    # ---- prologue: load K, build kt + knat ----
    for ko in range(KO):
        kc = ld_pool.tile([P, D], F32, tag="ld")
        nc.sync.dma_start(out=kc, in_=k_ap[ko * P:(ko + 1) * P, :])
        nc.vector.tensor_copy(out=knat[:, ko, :], in_=kc)  # round f32 -> f32r
        for half in range(2):
            trt = tr_psum.tile([P, 4 * P], F32, tag="tr")
            for j in range(4):
                do = half * 4 + j
                nc.tensor.transpose(
                    trt[:, j * P:(j + 1) * P], kc[:, do * P:(do + 1) * P], ident
                )
            nc.vector.tensor_copy(
                out=kt[:, half * 4:(half + 1) * 4, ko * P:(ko + 1) * P],
                in_=trt.rearrange("p (j f) -> p j f", j=4),
            )

    # ---- main loop over q blocks ----
    for rep in range(n_reps):
        for qb in range(QB):
            qc = ld_pool.tile([P, D], F32, tag="ld")
            nc.sync.dma_start(out=qc, in_=q_ap[qb * P:(qb + 1) * P, :])
            qt = qt_pool.tile([P, DO, P], F32R, tag="qt")
            for half in range(2):
                trt = tr_psum.tile([P, 4 * P], F32, tag="tr")
                for j in range(4):
                    do = half * 4 + j
                    nc.tensor.transpose(
                        trt[:, j * P:(j + 1) * P], qc[:, do * P:(do + 1) * P], ident
                    )
                nc.vector.tensor_copy(
                    out=qt[:, half * 4:(half + 1) * 4, :],
                    in_=trt.rearrange("p (j f) -> p j f", j=4),
                )

            # S = Q @ K^T : [128, 2048] in PSUM, accumulate over d-tiles
            s_ps = s_psum.tile([P, T], F32, tag="s")
            for do in range(DO):
                for kc4 in range(4):
                    nc.tensor.matmul(
                        s_ps[:, kc4 * 512:(kc4 + 1) * 512],
                        lhsT=qt[:, do, :],
                        rhs=kt[:, do, kc4 * 512:(kc4 + 1) * 512],
                        start=(do == 0),
                        stop=(do == DO - 1),
                    )

            # softmax over free dim
            negmax = stat_pool.tile([P, 1], F32, tag="negmax")
            nc.vector.tensor_reduce(
                out=negmax, in_=s_ps, axis=mybir.AxisListType.X,
                op=mybir.AluOpType.max, negate=True,
            )
            p_sb = p_pool.tile([P, T], F32, tag="p")
            sumexp = stat_pool.tile([P, 1], F32, tag="sumexp")
            nc.scalar.activation(
                out=p_sb, in_=s_ps, func=mybir.ActivationFunctionType.Exp,
                bias=negmax, scale=1.0, accum_out=sumexp,
            )
            recip = stat_pool.tile([P, 1], F32, tag="recip")
            nc.vector.reciprocal(recip, sumexp)

            # P^T tiles (f32r) for the second matmul
            pt = pt_pool.tile([P, KO, P], F32R, tag="pt")
            for quad in range(4):
                trt = tr_psum.tile([P, 4 * P], F32, tag="tr")
                for j in range(4):
                    ko = quad * 4 + j
                    nc.tensor.transpose(
                        trt[:, j * P:(j + 1) * P], p_sb[:, ko * P:(ko + 1) * P], ident
                    )
                nc.vector.tensor_copy(
                    out=pt[:, quad * 4:(quad + 1) * 4, :],
                    in_=trt.rearrange("p (j f) -> p j f", j=4),
                )

            # O = P @ K : [128, 1024] in PSUM, accumulate over k-tiles
            o_ps = o_psum.tile([P, D], F32, tag="o")
            for ko in range(KO):
                for c in range(2):
                    nc.tensor.matmul(
                        o_ps[:, c * 512:(c + 1) * 512],
                        lhsT=pt[:, ko, :],
                        rhs=knat[:, ko, c * 512:(c + 1) * 512],
                        start=(ko == 0),
                        stop=(ko == KO - 1),
                    )

            o_sb = o_pool.tile([P, D], F32, tag="o_sb")
            nc.vector.tensor_scalar_mul(o_sb, o_ps, recip)
            nc.sync.dma_start(out=out_ap[qb * P:(qb + 1) * P, :], in_=o_sb)


def build_nc(n_reps=1):
    from contextlib import ExitStack

    nc = bacc.Bacc("TRN2", target_bir_lowering=False, debug=False,
                   num_devices=N_CORES)
    q_ap = nc.dram_tensor("q", [T, D], F32, kind="ExternalInput").ap()
    k_ap = nc.dram_tensor("k", [T, D], F32, kind="ExternalInput").ap()
    out_ap = nc.dram_tensor("out", [T, D], F32, kind="ExternalOutput").ap()
    with tile.TileContext(nc) as tc:
        with ExitStack() as ctx:
            build_body(nc, tc, ctx, q_ap, k_ap, out_ap, n_reps=n_reps)
    nc.compile()
    return nc


_nc_cache = {}


def kernel(query: np.ndarray, key: np.ndarray) -> np.ndarray:
    """Full unsharded inputs [8, 2048, 1024] fp32 -> output [8, 2048, 1024]."""
    assert query.shape == (N_CORES, T, D) and key.shape == (N_CORES, T, D)
    if "nc" not in _nc_cache:
        _nc_cache["nc"] = build_nc()
    nc = _nc_cache["nc"]
    in_maps = [
        {"q": np.ascontiguousarray(query[b], dtype=np.float32),
         "k": np.ascontiguousarray(key[b], dtype=np.float32)}
        for b in range(N_CORES)
    ]
    res = run_bass_kernel_spmd(nc, in_maps, list(range(N_CORES)))
    out = np.stack([res.results[b]["out"] for b in range(N_CORES)], axis=0)
    return out.astype(np.float32)
